# revision 13
# baseline (speedup 1.0000x reference)
"""Trainium2 Bass kernel for nn_MultiHeadAttention_44908178047033.

T5-style MHA (relative-position bias, bidirectional) over
B=2, L=2048, D=768, H=12, DK=64.

Sharding: 8 cores = 2 batches x 4 head-groups (3 heads each).

v3 design:
- All matmuls bf16 (host casts q/kv/weights to bf16).
- Host pre-lays every DRAM tensor out in SBUF partition order so each
  DMA is 128 long contiguous lines (v2's 768-short-line DMAs clogged
  the sync queue for ~130us).
- Input DMA interleaved (wq, wk, then q/kv n-slices) so projections
  start ~3us in.
- Attention software-pipelined at half-step granularity (half-step x =
  (kc, jh) covers q cols [1024*jh, 1024*jh+1024)): scores for x+2 are
  emitted before PV of x so the in-order PE queue never blocks on the
  Scalar exp stream (the pacing engine, ~1.3us per [128,1024] tile).
- One exp per tile (bias = cm). The far-positive region (k-q >= 128,
  bias cp) is fixed after exp by a DVE tensor_scalar multiply with
  mu = e^(cp-cm) in bf16 2x mode.
- Near-diagonal Toeplitz bias added on DVE from the host-precomputed
  shifted table (negative free-dim stride).
- Per-head softmax normalization spread one op per half-step into the
  NEXT head's loop (denominator rows moved to partition 0 by tiny
  SBUF->SBUF DMAs since partition_broadcast reads partition 0; recip on
  DVE; broadcast + multiply on the otherwise-idle GpSimd), so the DVE
  queue never serializes at head boundaries.
- Output projection pairs heads 0+1 (C=128; h1's rows are partition-
  shifted by an SBUF->SBUF DMA), h2 is a second C=64 accumulation pass;
  h2's normalization is interleaved with the output-projection groups.
- Output partials stored bf16 (host sums in f32).
"""

import math
import sys
import threading

import numpy as np

sys.path.insert(0, "/opt/trn_rl_repo")

B, L, D = 2, 2048, 768
H, DK = 12, 64
NUM_BUCKETS, MAX_DIST = 32, 128
HP = 3            # heads per core
HD = HP * DK      # 192 cols per head-group
NCORES = 8
KC = 16           # key chunks of 128
NQ = 4            # q slices of 512
CCH = 6           # contraction chunks of 128 over D

_cache = {}
_lock = threading.Lock()


def _np_bucket(d):
    rel = d
    ret = np.zeros_like(rel)
    n = -rel
    nb = NUM_BUCKETS // 2
    ret = ret + (n < 0).astype(np.int32) * nb
    n = np.abs(n)
    mx = nb // 2
    is_small = n < mx
    n_safe = np.maximum(n, 1).astype(np.float32)
    vl = mx + (
        np.log(n_safe / mx) / math.log(MAX_DIST / mx) * (nb - mx)
    ).astype(np.int32)
    vl = np.minimum(vl, nb - 1)
    return ret + np.where(is_small, n, vl)


def _build_program():
    import concourse.bacc as bacc
    import concourse.bass as bass
    import concourse.mybir as mybir
    import concourse.tile as tile

    dt = mybir.dt
    f32, bf16 = dt.float32, dt.bfloat16

    nc = bacc.Bacc("TRN2", target_bir_lowering=False, debug=False,
                   num_devices=NCORES)

    # all host tensors are pre-laid-out [128 partitions, contiguous free]
    qT_d = nc.dram_tensor("qTn", [128, NQ, CCH, 512], bf16,
                          kind="ExternalInput").ap()
    kvT_d = nc.dram_tensor("kvTn", [128, NQ, CCH, 512], bf16,
                           kind="ExternalInput").ap()
    wq_d = nc.dram_tensor("wq", [128, CCH, HD], bf16, kind="ExternalInput").ap()
    wk_d = nc.dram_tensor("wk", [128, CCH, HD], bf16, kind="ExternalInput").ap()
    wv_d = nc.dram_tensor("wv", [128, CCH, HD], bf16, kind="ExternalInput").ap()
    wop_d = nc.dram_tensor("wop", [128, D], bf16, kind="ExternalInput").ap()
    wo2_d = nc.dram_tensor("wo2", [64, D], bf16, kind="ExternalInput").ap()
    sh_d = nc.dram_tensor("sh", [128, HP, 383], f32, kind="ExternalInput").ap()
    mskb_d = nc.dram_tensor("mskb", [128, KC], bf16, kind="ExternalInput").ap()
    cm_d = nc.dram_tensor("cm", [128, HP], f32, kind="ExternalInput").ap()
    mu_d = nc.dram_tensor("mu", [128, HP], f32, kind="ExternalInput").ap()
    out_d = nc.dram_tensor("out_p", [L, D], bf16, kind="ExternalOutput").ap()

    with tile.TileContext(nc) as tc:
        with (
            tc.tile_pool(name="const", bufs=1) as cpool,
            tc.tile_pool(name="pdyn", bufs=4) as pdyn,
            tc.tile_pool(name="ivb", bufs=3) as ivb,
            tc.tile_pool(name="den", bufs=5) as dpool,
            tc.tile_pool(name="ost", bufs=2) as ost,
            tc.tile_pool(name="sp", bufs=2, space="PSUM") as sp,
            tc.tile_pool(name="ap_", bufs=4, space="PSUM") as apool,
        ):
            # ---- persistent SBUF ----
            wq = cpool.tile([128, CCH, HD], bf16, tag="wq")
            wk = cpool.tile([128, CCH, HD], bf16, tag="wk")
            wv = cpool.tile([128, CCH, HD], bf16, tag="wv")
            wop = cpool.tile([128, D], bf16, tag="wop")
            wo2 = cpool.tile([64, D], bf16, tag="wo2")
            sh = cpool.tile([128, HP, 383], f32, tag="sh")
            mskb = cpool.tile([128, KC], bf16, tag="mskb")
            cmc = cpool.tile([128, HP], f32, tag="cmc")
            muc = cpool.tile([128, HP], f32, tag="muc")
            qT = cpool.tile([128, NQ, CCH, 512], bf16, tag="qT")
            kvT = cpool.tile([128, NQ, CCH, 512], bf16, tag="kvT")
            # heads 0,1 stacked on partitions 0-63 / 64-127; head 2 separate
            QTa = cpool.tile([128, L], bf16, tag="QTa")
            QTb = cpool.tile([64, L], bf16, tag="QTb")
            KTa = cpool.tile([128, L], bf16, tag="KTa")
            KTb = cpool.tile([64, L], bf16, tag="KTb")
            Vg = cpool.tile([128, KC, HP, 65], bf16, tag="Vg")
            # normalized attention outputs: ATa = h0 (p0-63) + h1 (p64-127)
            ATa = cpool.tile([128, L], bf16, tag="ATa")
            ATb = cpool.tile([64, L], bf16, tag="ATb")
            AT1t = cpool.tile([64, L], bf16, tag="AT1t")
            # PV results + denominators, all heads, f32
            pvsb = cpool.tile([65, HP, L], f32, tag="pvsb")

            # ---- loads, all on sync; first projection's inputs first ----
            nc.sync.dma_start(out=qT[:, 0], in_=qT_d[:, 0])
            nc.sync.dma_start(out=wq[:], in_=wq_d)
            nc.sync.dma_start(out=wk[:], in_=wk_d)
            nc.sync.dma_start(out=kvT[:, 0], in_=kvT_d[:, 0])
            for n in range(1, NQ):
                nc.sync.dma_start(out=qT[:, n], in_=qT_d[:, n])
                nc.sync.dma_start(out=kvT[:, n], in_=kvT_d[:, n])
            nc.sync.dma_start(out=wv[:], in_=wv_d)
            nc.sync.dma_start(out=wop[:], in_=wop_d)
            nc.sync.dma_start(out=wo2[:], in_=wo2_d)
            nc.sync.dma_start(out=sh[:], in_=sh_d)
            nc.sync.dma_start(out=mskb[:], in_=mskb_d)
            nc.sync.dma_start(out=cmc[:], in_=cm_d)
            nc.sync.dma_start(out=muc[:], in_=mu_d)

            # ---- Q/K projections, n-slice-major to chase the DMA ----
            for n in range(NQ):
                ns = slice(512 * n, 512 * n + 512)
                for w_in, x_in, dsts in ((wq, qT, (QTa, QTb)),
                                         (wk, kvT, (KTa, KTb))):
                    ps = sp.tile([128, 1024], f32, tag="sp",
                                 name=f"ps{w_in.name}_{n}")
                    for (mlo, mw, fo) in ((0, 128, 0), (128, 64, 512)):
                        for c in range(CCH):
                            nc.tensor.matmul(
                                ps[0:mw, fo:fo + 512],
                                lhsT=w_in[:, c, mlo:mlo + mw],
                                rhs=x_in[:, n, c, :],
                                start=(c == 0), stop=(c == CCH - 1),
                            )
                    nc.vector.tensor_copy(dsts[0][:, ns], ps[:, 0:512])
                    nc.vector.tensor_copy(dsts[1][:, ns], ps[0:64, 512:1024])

            # ---- V projection -> V_aug (bf16) with mask column ----
            for kc in range(KC):
                n, off = divmod(128 * kc, 512)
                ps_v = sp.tile([128, 1024], f32, tag="sp", name=f"psv{kc}")
                for c in range(CCH):
                    nc.tensor.matmul(
                        ps_v[:, 0:HD],
                        lhsT=kvT[:, n, c, off:off + 128],
                        rhs=wv[:, c, :],
                        start=(c == 0), stop=(c == CCH - 1),
                    )
                nc.vector.tensor_copy(
                    Vg[:, kc, :, 0:64],
                    ps_v[:, 0:HD].rearrange("p (h d) -> p h d", h=HP))
                mrep = bass.AP(mskb[:].tensor, mskb[:].offset + kc,
                               [list(mskb[:].ap[0]), [0, HP], [1, 1]])
                nc.vector.tensor_copy(Vg[:, kc, :, 64:65], mrep)

            def st_ops(h):
                """(lhsT_base, rhs_base) access helpers for head h."""
                if h == 0:
                    return (lambda kc: KTa[0:64, 128 * kc:128 * kc + 128],
                            lambda lo, w: QTa[0:64, lo:lo + w])
                if h == 1:
                    return (lambda kc: KTa[64:128, 128 * kc:128 * kc + 128],
                            lambda lo, w: QTa[64:128, lo:lo + w])
                return (lambda kc: KTb[0:64, 128 * kc:128 * kc + 128],
                        lambda lo, w: QTb[0:64, lo:lo + w])

            # ---- fused attention (S^T [k, q]), software-pipelined ----
            HSTEPS = KC * 2

            def score_step(h, x, kslice, qslice):
                kc, jh = divmod(x, 2)
                base = 1024 * jh
                s = sp.tile([128, 1024], f32, tag="sp", name=f"s{h}_{x}")
                for jj in range(2):
                    nc.tensor.matmul(
                        s[:, 512 * jj:512 * jj + 512],
                        lhsT=kslice(kc),
                        rhs=qslice(base + 512 * jj, 512),
                        start=True, stop=True,
                    )
                # near-diagonal bias add (in place, PSUM)
                qlo = max(0, 128 * kc - 128)
                qhi = min(L, 128 * kc + 255)
                a = max(qlo, base)
                b = min(qhi, base + 1024)
                if b > a:
                    sh_ap = sh[:, h, :]
                    x0 = (2047 + 128 * kc - qlo) - 1793
                    rev = bass.AP(
                        sh_ap.tensor, sh_ap.offset + x0 - (a - qlo),
                        [list(sh_ap.ap[0]), [-1, b - a]],
                    )
                    nc.vector.tensor_add(
                        s[:, a - base:b - base], s[:, a - base:b - base], rev)
                p = pdyn.tile([128, 1024], bf16, tag="pslab", name=f"p{h}_{x}")
                nc.scalar.activation(
                    p[:], s[:], mybir.ActivationFunctionType.Exp,
                    bias=cmc[:, h:h + 1], scale=1.0)
                return p

            def fix_step(h, x, p):
                # cp-region fixup: q < wcp has bias cp, not cm -> scale by mu
                kc, jh = divmod(x, 2)
                base = 1024 * jh
                wcp = max(0, 128 * kc - 128)
                wl = min(max(wcp - base, 0), 1024)
                if wl > 0:
                    nc.vector.tensor_scalar_mul(
                        p[:, 0:wl], p[:, 0:wl], muc[:, h:h + 1])

            def pv_step(h, x, p, accs, vsl):
                kc, jh = divmod(x, 2)
                for jj in range(2):
                    nc.tensor.matmul(
                        accs[2 * jh + jj][0:65, :],
                        lhsT=vsl(kc),
                        rhs=p[:, 512 * jj:512 * jj + 512],
                        start=(kc == 0), stop=(kc == KC - 1),
                    )

            def finish_head(h, accs):
                """Evict accumulators (PSUM WAR for the next head, on the
                otherwise-idle GpSimd so the DVE queue stays clear for the
                next head's bias adds) and move denominator rows to
                partition-0 tiles; returns deferred per-j normalization
                closures."""
                at_dst = (ATa if h == 0 else (AT1t if h == 1 else ATb))
                dens = [None] * NQ

                def evict_j(j):
                    jsl = slice(512 * j, 512 * j + 512)
                    nc.vector.tensor_copy(pvsb[:, h, jsl], accs[j][0:65, :])
                    dj = dpool.tile([1, 512], f32, tag="den",
                                    name=f"den{h}_{j}")
                    nc.sync.dma_start(out=dj[:], in_=pvsb[64:65, h, jsl])
                    dens[j] = dj

                # j0/j1 feed the next head's first PV (PSUM WAR) -> now;
                # j2/j3 are only needed by its second half-step -> deferred
                evict_j(0)
                evict_j(1)

                def norm_j(j):
                    jsl = slice(512 * j, 512 * j + 512)
                    inv = ivb.tile([64, 512], f32, tag="ivb",
                                   name=f"inv{h}_{j}")
                    nc.gpsimd.partition_broadcast(inv[:], dens[j][:])
                    # reciprocal on 64 partitions (a [1,512] recip runs on a
                    # single DVE lane and costs 3.3us)
                    nc.vector.reciprocal(inv[:], inv[:])
                    nc.gpsimd.tensor_mul(
                        at_dst[0:64, jsl], pvsb[0:64, h, jsl], inv[:])
                    if h == 1:
                        nc.sync.dma_start(out=ATa[64:128, jsl],
                                          in_=AT1t[:, jsl])
                return ([lambda j=j: evict_j(j) for j in (2, 3)] +
                        [lambda j=j: norm_j(j) for j in range(NQ)])

            pending = []
            for h in range(HP):
                kslice, qslice = st_ops(h)
                vsl = lambda kc: Vg[:, kc, h, :]
                accs = [apool.tile([128, 512], f32, tag="ap_",
                                   name=f"acc{h}_{j}")
                        for j in range(NQ)]
                ps = {}
                ps[0] = score_step(h, 0, kslice, qslice)
                ps[1] = score_step(h, 1, kslice, qslice)
                for x in range(HSTEPS):
                    # cp fixup first: exp(x) finished ~2 steps ago, so this
                    # DVE op runs immediately and PV(x) finds its dependency
                    # satisfied; the band add for x+2 queues right behind it
                    fix_step(h, x, ps[x])
                    if x + 2 < HSTEPS:
                        ps[x + 2] = score_step(h, x + 2, kslice, qslice)
                    if pending:
                        pending.pop(0)()
                    pv_step(h, x, ps.pop(x), accs, vsl)
                pending = finish_head(h, accs)

            # ---- tail: h2 normalization running ahead of the output
            # projection (per 512-col group) ----
            for _ in range(4):         # h2 evict j2/j3 + norm j0/j1
                pending.pop(0)()
            for g in range(NQ):
                for qc in range(4 * g, 4 * g + 4):
                    qsl = slice(128 * qc, 128 * qc + 128)
                    o = ost.tile([128, D], bf16, tag="ost", name=f"o{qc}")
                    for (nlo, nw) in ((0, 512), (512, 256)):
                        ps_o = apool.tile([128, 512], f32, tag="ap_",
                                          name=f"po{qc}_{nlo}")
                        nc.tensor.matmul(
                            ps_o[:, 0:nw],
                            lhsT=ATa[:, qsl],
                            rhs=wop[:, nlo:nlo + nw],
                            start=True, stop=False,
                        )
                        nc.tensor.matmul(
                            ps_o[:, 0:nw],
                            lhsT=ATb[:, qsl],
                            rhs=wo2[:, nlo:nlo + nw],
                            start=False, stop=True,
                        )
                        if (qc + (nlo > 0)) % 2 == 0:
                            nc.vector.tensor_copy(o[:, nlo:nlo + nw],
                                                  ps_o[:, 0:nw])
                        else:
                            nc.scalar.copy(o[:, nlo:nlo + nw], ps_o[:, 0:nw])
                    nc.sync.dma_start(out=out_d[qsl, :], in_=o[:])
                if pending:
                    pending.pop(0)()   # h2 norm j2/j3 ahead of its group

    nc.compile()
    return nc


def _get_program():
    with _lock:
        if "nc" not in _cache:
            _cache["nc"] = _build_program()
        return _cache["nc"]


def _host_prep(core, query, key_value, key_padding_mask, Wq, Wk, Wv, Wo, rel_emb):
    import ml_dtypes

    b, g = core // 4, core % 4
    bf = ml_dtypes.bfloat16
    mask = key_padding_mask[b].astype(np.float32)
    kv = key_value[b] * mask[:, None]
    # [128, NQ, CCH, 512]: partition p, n-slice, c-chunk (rows 128c+p)
    qTn = np.ascontiguousarray(
        query[b].T.reshape(CCH, 128, NQ, 512).transpose(1, 2, 0, 3)
    ).astype(bf)
    kvTn = np.ascontiguousarray(
        kv.T.reshape(CCH, 128, NQ, 512).transpose(1, 2, 0, 3)
    ).astype(bf)
    sl = slice(HD * g, HD * (g + 1))
    # [128, CCH, HD]: weight row 128c+p at [p, c, :]
    wq = np.ascontiguousarray(
        Wq[:, sl].reshape(CCH, 128, HD).transpose(1, 0, 2)).astype(bf)
    wk = np.ascontiguousarray(
        (Wk[:, sl] * np.float32(DK ** -0.5)).reshape(CCH, 128, HD)
        .transpose(1, 0, 2)).astype(bf)
    wv = np.ascontiguousarray(
        Wv[:, sl].reshape(CCH, 128, HD).transpose(1, 0, 2)).astype(bf)
    wo3 = Wo[sl].reshape(HP, 64, D)
    wop = np.ascontiguousarray(wo3[0:2].reshape(128, D)).astype(bf)
    wo2 = np.ascontiguousarray(wo3[2]).astype(bf)

    d = np.arange(-2047, 2048)
    buckets = _np_bucket(d)
    heads = [HP * g + i for i in range(HP)]
    t = rel_emb[buckets][:, heads].astype(np.float32)  # [4095, HP]
    cm = t[0]
    cp = t[-1]
    # sh[p, h, y] = t[y + 1793 + p, h] - cm[h]
    p_i = np.arange(128)[:, None]
    y_i = np.arange(383)[None, :]
    sh = np.ascontiguousarray(
        (t[y_i + 1793 + p_i] - cm[None, None, :]).transpose(0, 2, 1))
    msk = np.ascontiguousarray(mask.reshape(KC, 128).T)
    cmc = np.ascontiguousarray(np.broadcast_to(cm[None, :], (128, HP)))
    mu = np.exp(cp - cm).astype(np.float32)
    muc = np.ascontiguousarray(np.broadcast_to(mu[None, :], (128, HP)))
    return {
        "qTn": qTn, "kvTn": kvTn, "wq": wq, "wk": wk, "wv": wv,
        "wop": wop, "wo2": wo2,
        "sh": sh.astype(np.float32),
        "mskb": msk.astype(bf),
        "cm": cmc.astype(np.float32), "mu": muc.astype(np.float32),
    }


def make_in_maps(**inputs):
    return [_host_prep(c, **inputs) for c in range(NCORES)]


def kernel(query, key_value, key_padding_mask, Wq, Wk, Wv, Wo, rel_emb,
           _results_hook=None, _run_kwargs=None):
    from concourse.bass_utils import run_bass_kernel_spmd

    inputs = dict(query=np.asarray(query), key_value=np.asarray(key_value),
                  key_padding_mask=np.asarray(key_padding_mask),
                  Wq=np.asarray(Wq, np.float32), Wk=np.asarray(Wk, np.float32),
                  Wv=np.asarray(Wv, np.float32), Wo=np.asarray(Wo, np.float32),
                  rel_emb=np.asarray(rel_emb, np.float32))
    nc = _get_program()
    in_maps = make_in_maps(**inputs)
    res = run_bass_kernel_spmd(nc, in_maps, core_ids=list(range(NCORES)),
                               **(_run_kwargs or {}))
    if _results_hook is not None:
        _results_hook(res)
    out = np.zeros((B, L, D), np.float32)
    for c in range(NCORES):
        out[c // 4] += res.results[c]["out_p"].astype(np.float32)
    return out


# revision 15
# speedup vs baseline: 1.2001x; 1.2001x over previous
"""Trainium2 Bass kernel for nn_MultiHeadAttention_44908178047033.

T5-style MHA (relative-position bias, bidirectional) over
B=2, L=2048, D=768, H=12, DK=64.

Sharding: 8 cores = 2 batches x 4 head-groups (3 heads each).

v3 design:
- All matmuls bf16 (host casts q/kv/weights to bf16).
- Host pre-lays every DRAM tensor out in SBUF partition order so each
  DMA is 128 long contiguous lines (v2's 768-short-line DMAs clogged
  the sync queue for ~130us).
- Input DMA interleaved (wq, wk, then q/kv n-slices) so projections
  start ~3us in.
- Attention software-pipelined at half-step granularity (half-step x =
  (kc, jh) covers q cols [1024*jh, 1024*jh+1024)): scores for x+2 are
  emitted before PV of x so the in-order PE queue never blocks on the
  Scalar exp stream (the pacing engine, ~1.3us per [128,1024] tile).
- One exp per tile (bias = cm). The far-positive region (k-q >= 128,
  bias cp) is fixed after exp by a DVE tensor_scalar multiply with
  mu = e^(cp-cm) in bf16 2x mode.
- Near-diagonal Toeplitz bias added on DVE from the host-precomputed
  shifted table (negative free-dim stride).
- Per-head softmax normalization spread one op per half-step into the
  NEXT head's loop (denominator rows moved to partition 0 by tiny
  SBUF->SBUF DMAs since partition_broadcast reads partition 0; recip on
  DVE; broadcast + multiply on the otherwise-idle GpSimd), so the DVE
  queue never serializes at head boundaries.
- Output projection pairs heads 0+1 (C=128; h1's rows are partition-
  shifted by an SBUF->SBUF DMA), h2 is a second C=64 accumulation pass;
  h2's normalization is interleaved with the output-projection groups.
- Output partials stored bf16 (host sums in f32).
"""

import math
import sys
import threading

import numpy as np

sys.path.insert(0, "/opt/trn_rl_repo")

B, L, D = 2, 2048, 768
H, DK = 12, 64
NUM_BUCKETS, MAX_DIST = 32, 128
HP = 3            # heads per core
HD = HP * DK      # 192 cols per head-group
NCORES = 8
KC = 16           # key chunks of 128
NQ = 4            # q slices of 512
CCH = 6           # contraction chunks of 128 over D

_cache = {}
_lock = threading.Lock()


def _np_bucket(d):
    rel = d
    ret = np.zeros_like(rel)
    n = -rel
    nb = NUM_BUCKETS // 2
    ret = ret + (n < 0).astype(np.int32) * nb
    n = np.abs(n)
    mx = nb // 2
    is_small = n < mx
    n_safe = np.maximum(n, 1).astype(np.float32)
    vl = mx + (
        np.log(n_safe / mx) / math.log(MAX_DIST / mx) * (nb - mx)
    ).astype(np.int32)
    vl = np.minimum(vl, nb - 1)
    return ret + np.where(is_small, n, vl)


def _build_program():
    import concourse.bacc as bacc
    import concourse.bass as bass
    import concourse.mybir as mybir
    import concourse.tile as tile

    dt = mybir.dt
    f32, bf16 = dt.float32, dt.bfloat16

    nc = bacc.Bacc("TRN2", target_bir_lowering=False, debug=False,
                   num_devices=NCORES)

    # all host tensors are pre-laid-out [128 partitions, contiguous free]
    qT_d = nc.dram_tensor("qTn", [128, NQ, CCH, 512], bf16,
                          kind="ExternalInput").ap()
    kvT_d = nc.dram_tensor("kvTn", [128, NQ, CCH, 512], bf16,
                           kind="ExternalInput").ap()
    wq_d = nc.dram_tensor("wq", [128, CCH, HD], bf16, kind="ExternalInput").ap()
    wk_d = nc.dram_tensor("wk", [128, CCH, HD], bf16, kind="ExternalInput").ap()
    wv_d = nc.dram_tensor("wv", [128, CCH, HD], bf16, kind="ExternalInput").ap()
    wop_d = nc.dram_tensor("wop", [128, D], bf16, kind="ExternalInput").ap()
    wo2_d = nc.dram_tensor("wo2", [64, D], bf16, kind="ExternalInput").ap()
    sh_d = nc.dram_tensor("sh", [128, HP, 383], f32, kind="ExternalInput").ap()
    mskb_d = nc.dram_tensor("mskb", [128, KC], bf16, kind="ExternalInput").ap()
    cm_d = nc.dram_tensor("cm", [128, HP], f32, kind="ExternalInput").ap()
    mu_d = nc.dram_tensor("mu", [128, HP], f32, kind="ExternalInput").ap()
    out_d = nc.dram_tensor("out_p", [L, D], bf16, kind="ExternalOutput").ap()

    with tile.TileContext(nc) as tc:
        with (
            tc.tile_pool(name="const", bufs=1) as cpool,
            tc.tile_pool(name="pdyn", bufs=4) as pdyn,
            tc.tile_pool(name="ivb", bufs=3) as ivb,
            tc.tile_pool(name="den", bufs=5) as dpool,
            tc.tile_pool(name="ost", bufs=2) as ost,
            tc.tile_pool(name="sp", bufs=2, space="PSUM") as sp,
            tc.tile_pool(name="ap_", bufs=4, space="PSUM") as apool,
        ):
            # ---- persistent SBUF ----
            wq = cpool.tile([128, CCH, HD], bf16, tag="wq")
            wk = cpool.tile([128, CCH, HD], bf16, tag="wk")
            wv = cpool.tile([128, CCH, HD], bf16, tag="wv")
            wop = cpool.tile([128, D], bf16, tag="wop")
            wo2 = cpool.tile([64, D], bf16, tag="wo2")
            sh = cpool.tile([128, HP, 383], f32, tag="sh")
            mskb = cpool.tile([128, KC], bf16, tag="mskb")
            cmc = cpool.tile([128, HP], f32, tag="cmc")
            muc = cpool.tile([128, HP], f32, tag="muc")
            qT = cpool.tile([128, NQ, CCH, 512], bf16, tag="qT")
            kvT = cpool.tile([128, NQ, CCH, 512], bf16, tag="kvT")
            # heads 0,1 stacked on partitions 0-63 / 64-127; head 2 separate
            QTa = cpool.tile([128, L], bf16, tag="QTa")
            QTb = cpool.tile([64, L], bf16, tag="QTb")
            KTa = cpool.tile([128, L], bf16, tag="KTa")
            KTb = cpool.tile([64, L], bf16, tag="KTb")
            Vg = cpool.tile([128, KC, HP, 65], bf16, tag="Vg")
            # normalized attention outputs: ATa = h0 (p0-63) + h1 (p64-127)
            ATa = cpool.tile([128, L], bf16, tag="ATa")
            ATb = cpool.tile([64, L], bf16, tag="ATb")
            AT1t = cpool.tile([64, L], bf16, tag="AT1t")
            # PV results + denominators, all heads, f32
            pvsb = cpool.tile([65, HP, L], f32, tag="pvsb")

            # ---- loads, all on sync; first projection's inputs first ----
            nc.sync.dma_start(out=qT[:, 0], in_=qT_d[:, 0])
            nc.sync.dma_start(out=wq[:], in_=wq_d)
            nc.sync.dma_start(out=wk[:], in_=wk_d)
            nc.sync.dma_start(out=kvT[:, 0], in_=kvT_d[:, 0])
            for n in range(1, NQ):
                nc.sync.dma_start(out=qT[:, n], in_=qT_d[:, n])
                nc.sync.dma_start(out=kvT[:, n], in_=kvT_d[:, n])
            nc.sync.dma_start(out=wv[:], in_=wv_d)
            nc.sync.dma_start(out=wop[:], in_=wop_d)
            nc.sync.dma_start(out=wo2[:], in_=wo2_d)
            nc.sync.dma_start(out=sh[:], in_=sh_d)
            nc.sync.dma_start(out=mskb[:], in_=mskb_d)
            nc.sync.dma_start(out=cmc[:], in_=cm_d)
            nc.sync.dma_start(out=muc[:], in_=mu_d)

            # ---- Q/K projections, n-slice-major to chase the DMA ----
            for n in range(NQ):
                ns = slice(512 * n, 512 * n + 512)
                for w_in, x_in, dsts in ((wq, qT, (QTa, QTb)),
                                         (wk, kvT, (KTa, KTb))):
                    ps = sp.tile([128, 1024], f32, tag="sp",
                                 name=f"ps{w_in.name}_{n}")
                    for (mlo, mw, fo) in ((0, 128, 0), (128, 64, 512)):
                        for c in range(CCH):
                            nc.tensor.matmul(
                                ps[0:mw, fo:fo + 512],
                                lhsT=w_in[:, c, mlo:mlo + mw],
                                rhs=x_in[:, n, c, :],
                                start=(c == 0), stop=(c == CCH - 1),
                            )
                    nc.vector.tensor_copy(dsts[0][:, ns], ps[:, 0:512])
                    nc.vector.tensor_copy(dsts[1][:, ns], ps[0:64, 512:1024])

            # ---- V projection -> V_aug (bf16) with mask column ----
            for kc in range(KC):
                n, off = divmod(128 * kc, 512)
                ps_v = sp.tile([128, 1024], f32, tag="sp", name=f"psv{kc}")
                for c in range(CCH):
                    nc.tensor.matmul(
                        ps_v[:, 0:HD],
                        lhsT=kvT[:, n, c, off:off + 128],
                        rhs=wv[:, c, :],
                        start=(c == 0), stop=(c == CCH - 1),
                    )
                nc.vector.tensor_copy(
                    Vg[:, kc, :, 0:64],
                    ps_v[:, 0:HD].rearrange("p (h d) -> p h d", h=HP))
                mrep = bass.AP(mskb[:].tensor, mskb[:].offset + kc,
                               [list(mskb[:].ap[0]), [0, HP], [1, 1]])
                nc.vector.tensor_copy(Vg[:, kc, :, 64:65], mrep)

            def st_ops(h):
                """(lhsT_base, rhs_base) access helpers for head h."""
                if h == 0:
                    return (lambda kc: KTa[0:64, 128 * kc:128 * kc + 128],
                            lambda lo, w: QTa[0:64, lo:lo + w])
                if h == 1:
                    return (lambda kc: KTa[64:128, 128 * kc:128 * kc + 128],
                            lambda lo, w: QTa[64:128, lo:lo + w])
                return (lambda kc: KTb[0:64, 128 * kc:128 * kc + 128],
                        lambda lo, w: QTb[0:64, lo:lo + w])

            # ---- fused attention (S^T [k, q]), software-pipelined ----
            HSTEPS = KC * 2

            def score_step(h, x, kslice, qslice):
                kc, jh = divmod(x, 2)
                base = 1024 * jh
                s = sp.tile([128, 1024], f32, tag="sp", name=f"s{h}_{x}")
                for jj in range(2):
                    nc.tensor.matmul(
                        s[:, 512 * jj:512 * jj + 512],
                        lhsT=kslice(kc),
                        rhs=qslice(base + 512 * jj, 512),
                        start=True, stop=True,
                    )
                # near-diagonal bias add (in place, PSUM)
                qlo = max(0, 128 * kc - 128)
                qhi = min(L, 128 * kc + 255)
                a = max(qlo, base)
                b = min(qhi, base + 1024)
                if b > a:
                    sh_ap = sh[:, h, :]
                    x0 = (2047 + 128 * kc - qlo) - 1793
                    rev = bass.AP(
                        sh_ap.tensor, sh_ap.offset + x0 - (a - qlo),
                        [list(sh_ap.ap[0]), [-1, b - a]],
                    )
                    nc.vector.tensor_add(
                        s[:, a - base:b - base], s[:, a - base:b - base], rev)
                p = pdyn.tile([128, 1024], bf16, tag="pslab", name=f"p{h}_{x}")
                nc.scalar.activation(
                    p[:], s[:], mybir.ActivationFunctionType.Exp,
                    bias=cmc[:, h:h + 1], scale=1.0)
                return p

            def fix_step(h, x, p):
                # cp-region fixup: q < wcp has bias cp, not cm -> scale by mu
                kc, jh = divmod(x, 2)
                base = 1024 * jh
                wcp = max(0, 128 * kc - 128)
                wl = min(max(wcp - base, 0), 1024)
                if wl > 0:
                    nc.vector.tensor_scalar_mul(
                        p[:, 0:wl], p[:, 0:wl], muc[:, h:h + 1])

            def pv_step(h, x, p, accs, vsl):
                kc, jh = divmod(x, 2)
                for jj in range(2):
                    nc.tensor.matmul(
                        accs[2 * jh + jj][0:65, :],
                        lhsT=vsl(kc),
                        rhs=p[:, 512 * jj:512 * jj + 512],
                        start=(kc == 0), stop=(kc == KC - 1),
                    )

            def finish_head(h, accs):
                """Evict accumulators (PSUM WAR for the next head, on the
                otherwise-idle GpSimd so the DVE queue stays clear for the
                next head's bias adds) and move denominator rows to
                partition-0 tiles; returns deferred per-j normalization
                closures."""
                at_dst = (ATa if h == 0 else (AT1t if h == 1 else ATb))
                dens = [None] * NQ

                def evict_j(j):
                    jsl = slice(512 * j, 512 * j + 512)
                    nc.vector.tensor_copy(pvsb[:, h, jsl], accs[j][0:65, :])
                    dj = dpool.tile([1, 512], f32, tag="den",
                                    name=f"den{h}_{j}")
                    nc.sync.dma_start(out=dj[:], in_=pvsb[64:65, h, jsl])
                    dens[j] = dj

                # j0/j1 feed the next head's first PV (PSUM WAR) -> now;
                # j2/j3 are only needed by its second half-step -> deferred
                evict_j(0)
                evict_j(1)

                def norm_j(j):
                    jsl = slice(512 * j, 512 * j + 512)
                    # approx reciprocal (18 bits, ~5x faster than full
                    # precision on this single-partition tile); dependency
                    # (den DMA) is long satisfied so the DVE queue never
                    # blocks cross-engine here
                    nc.vector.reciprocal_approx_fast(dens[j][:], dens[j][:])
                    inv = ivb.tile([64, 512], f32, tag="ivb",
                                   name=f"inv{h}_{j}")
                    nc.gpsimd.partition_broadcast(inv[:], dens[j][:])
                    nc.gpsimd.tensor_mul(
                        at_dst[0:64, jsl], pvsb[0:64, h, jsl], inv[:])
                return ([lambda j=j: evict_j(j) for j in (2, 3)] +
                        [lambda j=j: norm_j(j) for j in range(NQ)])

            pending = []
            for h in range(HP):
                kslice, qslice = st_ops(h)
                vsl = lambda kc: Vg[:, kc, h, :]
                accs = [apool.tile([128, 512], f32, tag="ap_",
                                   name=f"acc{h}_{j}")
                        for j in range(NQ)]
                ps = {}
                ps[0] = score_step(h, 0, kslice, qslice)
                ps[1] = score_step(h, 1, kslice, qslice)
                for x in range(HSTEPS):
                    # cp fixup first: exp(x) finished ~2 steps ago, so this
                    # DVE op runs immediately and PV(x) finds its dependency
                    # satisfied; the band add for x+2 queues right behind it
                    fix_step(h, x, ps[x])
                    if x + 2 < HSTEPS:
                        ps[x + 2] = score_step(h, x + 2, kslice, qslice)
                    if pending:
                        pending.pop(0)()
                    pv_step(h, x, ps.pop(x), accs, vsl)
                pending = finish_head(h, accs)

            # ---- tail: h2 normalization running ahead of the output
            # projection (per 512-col group) ----
            for _ in range(4):         # h2 evict j2/j3 + norm j0/j1
                pending.pop(0)()
            # h1's rows to partitions 64..127 for the paired projection
            # (h1's norms completed during h2's pipeline)
            nc.sync.dma_start(out=ATa[64:128, :], in_=AT1t[:])
            for g in range(NQ):
                for qc in range(4 * g, 4 * g + 4):
                    qsl = slice(128 * qc, 128 * qc + 128)
                    o = ost.tile([128, D], bf16, tag="ost", name=f"o{qc}")
                    for (nlo, nw) in ((0, 512), (512, 256)):
                        ps_o = apool.tile([128, 512], f32, tag="ap_",
                                          name=f"po{qc}_{nlo}")
                        nc.tensor.matmul(
                            ps_o[:, 0:nw],
                            lhsT=ATa[:, qsl],
                            rhs=wop[:, nlo:nlo + nw],
                            start=True, stop=False,
                        )
                        nc.tensor.matmul(
                            ps_o[:, 0:nw],
                            lhsT=ATb[:, qsl],
                            rhs=wo2[:, nlo:nlo + nw],
                            start=False, stop=True,
                        )
                        if (qc + (nlo > 0)) % 2 == 0:
                            nc.vector.tensor_copy(o[:, nlo:nlo + nw],
                                                  ps_o[:, 0:nw])
                        else:
                            nc.scalar.copy(o[:, nlo:nlo + nw], ps_o[:, 0:nw])
                    nc.sync.dma_start(out=out_d[qsl, :], in_=o[:])
                if pending:
                    pending.pop(0)()   # h2 norm j2/j3 ahead of its group

    nc.compile()
    return nc


def _get_program():
    with _lock:
        if "nc" not in _cache:
            _cache["nc"] = _build_program()
        return _cache["nc"]


def _host_prep(core, query, key_value, key_padding_mask, Wq, Wk, Wv, Wo, rel_emb):
    import ml_dtypes

    b, g = core // 4, core % 4
    bf = ml_dtypes.bfloat16
    mask = key_padding_mask[b].astype(np.float32)
    kv = key_value[b] * mask[:, None]
    # [128, NQ, CCH, 512]: partition p, n-slice, c-chunk (rows 128c+p)
    qTn = np.ascontiguousarray(
        query[b].T.reshape(CCH, 128, NQ, 512).transpose(1, 2, 0, 3)
    ).astype(bf)
    kvTn = np.ascontiguousarray(
        kv.T.reshape(CCH, 128, NQ, 512).transpose(1, 2, 0, 3)
    ).astype(bf)
    sl = slice(HD * g, HD * (g + 1))
    # [128, CCH, HD]: weight row 128c+p at [p, c, :]
    wq = np.ascontiguousarray(
        Wq[:, sl].reshape(CCH, 128, HD).transpose(1, 0, 2)).astype(bf)
    wk = np.ascontiguousarray(
        (Wk[:, sl] * np.float32(DK ** -0.5)).reshape(CCH, 128, HD)
        .transpose(1, 0, 2)).astype(bf)
    wv = np.ascontiguousarray(
        Wv[:, sl].reshape(CCH, 128, HD).transpose(1, 0, 2)).astype(bf)
    wo3 = Wo[sl].reshape(HP, 64, D)
    wop = np.ascontiguousarray(wo3[0:2].reshape(128, D)).astype(bf)
    wo2 = np.ascontiguousarray(wo3[2]).astype(bf)

    d = np.arange(-2047, 2048)
    buckets = _np_bucket(d)
    heads = [HP * g + i for i in range(HP)]
    t = rel_emb[buckets][:, heads].astype(np.float32)  # [4095, HP]
    cm = t[0]
    cp = t[-1]
    # sh[p, h, y] = t[y + 1793 + p, h] - cm[h]
    p_i = np.arange(128)[:, None]
    y_i = np.arange(383)[None, :]
    sh = np.ascontiguousarray(
        (t[y_i + 1793 + p_i] - cm[None, None, :]).transpose(0, 2, 1))
    msk = np.ascontiguousarray(mask.reshape(KC, 128).T)
    cmc = np.ascontiguousarray(np.broadcast_to(cm[None, :], (128, HP)))
    mu = np.exp(cp - cm).astype(np.float32)
    muc = np.ascontiguousarray(np.broadcast_to(mu[None, :], (128, HP)))
    return {
        "qTn": qTn, "kvTn": kvTn, "wq": wq, "wk": wk, "wv": wv,
        "wop": wop, "wo2": wo2,
        "sh": sh.astype(np.float32),
        "mskb": msk.astype(bf),
        "cm": cmc.astype(np.float32), "mu": muc.astype(np.float32),
    }


def make_in_maps(**inputs):
    return [_host_prep(c, **inputs) for c in range(NCORES)]


def kernel(query, key_value, key_padding_mask, Wq, Wk, Wv, Wo, rel_emb,
           _results_hook=None, _run_kwargs=None):
    from concourse.bass_utils import run_bass_kernel_spmd

    inputs = dict(query=np.asarray(query), key_value=np.asarray(key_value),
                  key_padding_mask=np.asarray(key_padding_mask),
                  Wq=np.asarray(Wq, np.float32), Wk=np.asarray(Wk, np.float32),
                  Wv=np.asarray(Wv, np.float32), Wo=np.asarray(Wo, np.float32),
                  rel_emb=np.asarray(rel_emb, np.float32))
    nc = _get_program()
    in_maps = make_in_maps(**inputs)
    res = run_bass_kernel_spmd(nc, in_maps, core_ids=list(range(NCORES)),
                               **(_run_kwargs or {}))
    if _results_hook is not None:
        _results_hook(res)
    out = np.zeros((B, L, D), np.float32)
    for c in range(NCORES):
        out[c // 4] += res.results[c]["out_p"].astype(np.float32)
    return out


# revision 17
# speedup vs baseline: 1.2023x; 1.0019x over previous
"""Trainium2 Bass kernel for nn_MultiHeadAttention_44908178047033.

T5-style MHA (relative-position bias, bidirectional) over
B=2, L=2048, D=768, H=12, DK=64.

Sharding: 8 cores = 2 batches x 4 head-groups (3 heads each).

v3 design:
- All matmuls bf16 (host casts q/kv/weights to bf16).
- Host pre-lays every DRAM tensor out in SBUF partition order so each
  DMA is 128 long contiguous lines (v2's 768-short-line DMAs clogged
  the sync queue for ~130us).
- Input DMA interleaved (wq, wk, then q/kv n-slices) so projections
  start ~3us in.
- Attention software-pipelined at half-step granularity (half-step x =
  (kc, jh) covers q cols [1024*jh, 1024*jh+1024)): scores for x+2 are
  emitted before PV of x so the in-order PE queue never blocks on the
  Scalar exp stream (the pacing engine, ~1.3us per [128,1024] tile).
- One exp per tile (bias = cm). The far-positive region (k-q >= 128,
  bias cp) is fixed after exp by a DVE tensor_scalar multiply with
  mu = e^(cp-cm) in bf16 2x mode.
- Near-diagonal Toeplitz bias added on DVE from the host-precomputed
  shifted table (negative free-dim stride).
- Per-head softmax normalization spread one op per half-step into the
  NEXT head's loop (denominator rows moved to partition 0 by tiny
  SBUF->SBUF DMAs since partition_broadcast reads partition 0; recip on
  DVE; broadcast + multiply on the otherwise-idle GpSimd), so the DVE
  queue never serializes at head boundaries.
- Output projection pairs heads 0+1 (C=128; h1's rows are partition-
  shifted by an SBUF->SBUF DMA), h2 is a second C=64 accumulation pass;
  h2's normalization is interleaved with the output-projection groups.
- Output partials stored bf16 (host sums in f32).
"""

import math
import sys
import threading

import numpy as np

sys.path.insert(0, "/opt/trn_rl_repo")

B, L, D = 2, 2048, 768
H, DK = 12, 64
NUM_BUCKETS, MAX_DIST = 32, 128
HP = 3            # heads per core
HD = HP * DK      # 192 cols per head-group
NCORES = 8
KC = 16           # key chunks of 128
NQ = 4            # q slices of 512
CCH = 6           # contraction chunks of 128 over D

_cache = {}
_lock = threading.Lock()


def _np_bucket(d):
    rel = d
    ret = np.zeros_like(rel)
    n = -rel
    nb = NUM_BUCKETS // 2
    ret = ret + (n < 0).astype(np.int32) * nb
    n = np.abs(n)
    mx = nb // 2
    is_small = n < mx
    n_safe = np.maximum(n, 1).astype(np.float32)
    vl = mx + (
        np.log(n_safe / mx) / math.log(MAX_DIST / mx) * (nb - mx)
    ).astype(np.int32)
    vl = np.minimum(vl, nb - 1)
    return ret + np.where(is_small, n, vl)


def _build_program():
    import concourse.bacc as bacc
    import concourse.bass as bass
    import concourse.mybir as mybir
    import concourse.tile as tile

    dt = mybir.dt
    f32, bf16 = dt.float32, dt.bfloat16

    nc = bacc.Bacc("TRN2", target_bir_lowering=False, debug=False,
                   num_devices=NCORES)

    # all host tensors are pre-laid-out [128 partitions, contiguous free]
    qT_d = nc.dram_tensor("qTn", [128, NQ, CCH, 512], bf16,
                          kind="ExternalInput").ap()
    kvT_d = nc.dram_tensor("kvTn", [128, NQ, CCH, 512], bf16,
                           kind="ExternalInput").ap()
    wq_d = nc.dram_tensor("wq", [128, CCH, HD], bf16, kind="ExternalInput").ap()
    wk_d = nc.dram_tensor("wk", [128, CCH, HD], bf16, kind="ExternalInput").ap()
    wv_d = nc.dram_tensor("wv", [128, CCH, HD], bf16, kind="ExternalInput").ap()
    wop_d = nc.dram_tensor("wop", [128, D], bf16, kind="ExternalInput").ap()
    wo2_d = nc.dram_tensor("wo2", [64, D], bf16, kind="ExternalInput").ap()
    sh_d = nc.dram_tensor("sh", [128, HP, 383], f32, kind="ExternalInput").ap()
    mskb_d = nc.dram_tensor("mskb", [128, KC], bf16, kind="ExternalInput").ap()
    cm_d = nc.dram_tensor("cm", [128, HP], f32, kind="ExternalInput").ap()
    mu_d = nc.dram_tensor("mu", [128, HP], f32, kind="ExternalInput").ap()
    out_d = nc.dram_tensor("out_p", [L, D], bf16, kind="ExternalOutput").ap()

    with tile.TileContext(nc) as tc:
        with (
            tc.tile_pool(name="const", bufs=1) as cpool,
            tc.tile_pool(name="pdyn", bufs=4) as pdyn,
            tc.tile_pool(name="ivb", bufs=3) as ivb,
            tc.tile_pool(name="den", bufs=5) as dpool,
            tc.tile_pool(name="ost", bufs=2) as ost,
            tc.tile_pool(name="sp", bufs=2, space="PSUM") as sp,
            tc.tile_pool(name="ap_", bufs=4, space="PSUM") as apool,
        ):
            # ---- persistent SBUF ----
            wq = cpool.tile([128, CCH, HD], bf16, tag="wq")
            wk = cpool.tile([128, CCH, HD], bf16, tag="wk")
            wv = cpool.tile([128, CCH, HD], bf16, tag="wv")
            wop = cpool.tile([128, D], bf16, tag="wop")
            wo2 = cpool.tile([64, D], bf16, tag="wo2")
            sh = cpool.tile([128, HP, 383], f32, tag="sh")
            mskb = cpool.tile([128, KC], bf16, tag="mskb")
            cmc = cpool.tile([128, HP], f32, tag="cmc")
            muc = cpool.tile([128, HP], f32, tag="muc")
            qT = cpool.tile([128, NQ, CCH, 512], bf16, tag="qT")
            kvT = cpool.tile([128, NQ, CCH, 512], bf16, tag="kvT")
            # heads 0,1 stacked on partitions 0-63 / 64-127; head 2 separate
            QTa = cpool.tile([128, L], bf16, tag="QTa")
            QTb = cpool.tile([64, L], bf16, tag="QTb")
            KTa = cpool.tile([128, L], bf16, tag="KTa")
            KTb = cpool.tile([64, L], bf16, tag="KTb")
            Vg = cpool.tile([128, KC, HP, 65], bf16, tag="Vg")
            # normalized attention outputs: ATa = h0 (p0-63) + h1 (p64-127)
            ATa = cpool.tile([128, L], bf16, tag="ATa")
            ATb = cpool.tile([64, L], bf16, tag="ATb")
            AT1t = cpool.tile([64, L], bf16, tag="AT1t")
            # PV results + denominators, all heads, f32
            pvsb = cpool.tile([65, HP, L], f32, tag="pvsb")

            # ---- loads, all on sync; first projection's inputs first ----
            nc.sync.dma_start(out=qT[:, 0], in_=qT_d[:, 0])
            nc.sync.dma_start(out=wq[:], in_=wq_d)
            nc.sync.dma_start(out=wk[:], in_=wk_d)
            nc.sync.dma_start(out=kvT[:, 0], in_=kvT_d[:, 0])
            for n in range(1, NQ):
                nc.sync.dma_start(out=qT[:, n], in_=qT_d[:, n])
                nc.sync.dma_start(out=kvT[:, n], in_=kvT_d[:, n])
            nc.sync.dma_start(out=wv[:], in_=wv_d)
            nc.sync.dma_start(out=wop[:], in_=wop_d)
            nc.sync.dma_start(out=wo2[:], in_=wo2_d)
            nc.sync.dma_start(out=sh[:], in_=sh_d)
            nc.sync.dma_start(out=mskb[:], in_=mskb_d)
            nc.sync.dma_start(out=cmc[:], in_=cm_d)
            nc.sync.dma_start(out=muc[:], in_=mu_d)

            # ---- Q/K projections, n-slice-major to chase the DMA ----
            for n in range(NQ):
                ns = slice(512 * n, 512 * n + 512)
                for w_in, x_in, dsts in ((wq, qT, (QTa, QTb)),
                                         (wk, kvT, (KTa, KTb))):
                    ps = sp.tile([128, 1024], f32, tag="sp",
                                 name=f"ps{w_in.name}_{n}")
                    for (mlo, mw, fo) in ((0, 128, 0), (128, 64, 512)):
                        for c in range(CCH):
                            nc.tensor.matmul(
                                ps[0:mw, fo:fo + 512],
                                lhsT=w_in[:, c, mlo:mlo + mw],
                                rhs=x_in[:, n, c, :],
                                start=(c == 0), stop=(c == CCH - 1),
                            )
                    nc.vector.tensor_copy(dsts[0][:, ns], ps[:, 0:512])
                    nc.vector.tensor_copy(dsts[1][:, ns], ps[0:64, 512:1024])

            # ---- V projection -> V_aug (bf16) with mask column ----
            for kc in range(KC):
                n, off = divmod(128 * kc, 512)
                ps_v = sp.tile([128, 1024], f32, tag="sp", name=f"psv{kc}")
                for c in range(CCH):
                    nc.tensor.matmul(
                        ps_v[:, 0:HD],
                        lhsT=kvT[:, n, c, off:off + 128],
                        rhs=wv[:, c, :],
                        start=(c == 0), stop=(c == CCH - 1),
                    )
                nc.vector.tensor_copy(
                    Vg[:, kc, :, 0:64],
                    ps_v[:, 0:HD].rearrange("p (h d) -> p h d", h=HP))
                mrep = bass.AP(mskb[:].tensor, mskb[:].offset + kc,
                               [list(mskb[:].ap[0]), [0, HP], [1, 1]])
                nc.vector.tensor_copy(Vg[:, kc, :, 64:65], mrep)

            def st_ops(h):
                """(lhsT_base, rhs_base) access helpers for head h."""
                if h == 0:
                    return (lambda kc: KTa[0:64, 128 * kc:128 * kc + 128],
                            lambda lo, w: QTa[0:64, lo:lo + w])
                if h == 1:
                    return (lambda kc: KTa[64:128, 128 * kc:128 * kc + 128],
                            lambda lo, w: QTa[64:128, lo:lo + w])
                return (lambda kc: KTb[0:64, 128 * kc:128 * kc + 128],
                        lambda lo, w: QTb[0:64, lo:lo + w])

            # ---- fused attention (S^T [k, q]), software-pipelined ----
            HSTEPS = KC * 2

            def score_step(h, x, kslice, qslice):
                kc, jh = divmod(x, 2)
                base = 1024 * jh
                s = sp.tile([128, 1024], f32, tag="sp", name=f"s{h}_{x}")
                for jj in range(2):
                    nc.tensor.matmul(
                        s[:, 512 * jj:512 * jj + 512],
                        lhsT=kslice(kc),
                        rhs=qslice(base + 512 * jj, 512),
                        start=True, stop=True,
                    )
                # near-diagonal bias add (in place, PSUM)
                qlo = max(0, 128 * kc - 128)
                qhi = min(L, 128 * kc + 255)
                a = max(qlo, base)
                b = min(qhi, base + 1024)
                if b > a:
                    sh_ap = sh[:, h, :]
                    x0 = (2047 + 128 * kc - qlo) - 1793
                    rev = bass.AP(
                        sh_ap.tensor, sh_ap.offset + x0 - (a - qlo),
                        [list(sh_ap.ap[0]), [-1, b - a]],
                    )
                    nc.vector.tensor_add(
                        s[:, a - base:b - base], s[:, a - base:b - base], rev)
                p = pdyn.tile([128, 1024], bf16, tag="pslab", name=f"p{h}_{x}")
                nc.scalar.activation(
                    p[:], s[:], mybir.ActivationFunctionType.Exp,
                    bias=cmc[:, h:h + 1], scale=1.0)
                return p

            def fix_step(h, x, p):
                # cp-region fixup: q < wcp has bias cp, not cm -> scale by mu
                kc, jh = divmod(x, 2)
                base = 1024 * jh
                wcp = max(0, 128 * kc - 128)
                wl = min(max(wcp - base, 0), 1024)
                if wl > 0:
                    nc.vector.tensor_scalar_mul(
                        p[:, 0:wl], p[:, 0:wl], muc[:, h:h + 1])

            def pv_step(h, x, p, accs, vsl):
                kc, jh = divmod(x, 2)
                for jj in range(2):
                    nc.tensor.matmul(
                        accs[2 * jh + jj][0:65, :],
                        lhsT=vsl(kc),
                        rhs=p[:, 512 * jj:512 * jj + 512],
                        start=(kc == 0), stop=(kc == KC - 1),
                    )

            def finish_head(h, accs):
                """Evict accumulators (PSUM WAR for the next head, on the
                otherwise-idle GpSimd so the DVE queue stays clear for the
                next head's bias adds) and move denominator rows to
                partition-0 tiles; returns deferred per-j normalization
                closures."""
                at_dst = (ATa if h == 0 else (AT1t if h == 1 else ATb))
                dens = [None] * NQ

                def evict_j(j):
                    jsl = slice(512 * j, 512 * j + 512)
                    nc.vector.tensor_copy(pvsb[:, h, jsl], accs[j][0:65, :])
                    dj = dpool.tile([1, 512], f32, tag="den",
                                    name=f"den{h}_{j}")
                    nc.sync.dma_start(out=dj[:], in_=pvsb[64:65, h, jsl])
                    dens[j] = dj

                # j0/j1 feed the next head's first PV (PSUM WAR) -> now;
                # j2/j3 are only needed by its second half-step -> deferred
                evict_j(0)
                evict_j(1)

                def norm_j(j):
                    jsl = slice(512 * j, 512 * j + 512)
                    # approx reciprocal (18 bits, ~5x faster than full
                    # precision on this single-partition tile); dependency
                    # (den DMA) is long satisfied so the DVE queue never
                    # blocks cross-engine here
                    nc.vector.reciprocal_approx_fast(dens[j][:], dens[j][:])
                    inv = ivb.tile([64, 512], f32, tag="ivb",
                                   name=f"inv{h}_{j}")
                    nc.gpsimd.partition_broadcast(inv[:], dens[j][:])
                    nc.gpsimd.tensor_mul(
                        at_dst[0:64, jsl], pvsb[0:64, h, jsl], inv[:])
                return ([lambda j=j: evict_j(j) for j in (2, 3)] +
                        [lambda j=j: norm_j(j) for j in range(NQ)])

            pending = []
            for h in range(HP):
                kslice, qslice = st_ops(h)
                vsl = lambda kc: Vg[:, kc, h, :]
                accs = [apool.tile([128, 512], f32, tag="ap_",
                                   name=f"acc{h}_{j}")
                        for j in range(NQ)]
                ps = {}
                ps[0] = score_step(h, 0, kslice, qslice)
                ps[1] = score_step(h, 1, kslice, qslice)
                for x in range(HSTEPS):
                    # cp fixup first: exp(x) finished ~2 steps ago, so this
                    # DVE op runs immediately and PV(x) finds its dependency
                    # satisfied; the band add for x+2 queues right behind it
                    fix_step(h, x, ps[x])
                    if x + 2 < HSTEPS:
                        ps[x + 2] = score_step(h, x + 2, kslice, qslice)
                    if pending:
                        pending.pop(0)()
                    pv_step(h, x, ps.pop(x), accs, vsl)
                pending = finish_head(h, accs)

            # ---- tail: emit ALL h2 normalization chains upfront (they are
            # independent; each engine queue streams its 4 ops while the
            # output projection's semaphores pick groups up as they land) ----
            while pending:
                pending.pop(0)()
            # h1's rows to partitions 64..127 for the paired projection
            # (h1's norms completed during h2's pipeline)
            nc.sync.dma_start(out=ATa[64:128, :], in_=AT1t[:])
            for g in range(NQ):
                for qc in range(4 * g, 4 * g + 4):
                    qsl = slice(128 * qc, 128 * qc + 128)
                    o = ost.tile([128, D], bf16, tag="ost", name=f"o{qc}")
                    for (nlo, nw) in ((0, 512), (512, 256)):
                        ps_o = apool.tile([128, 512], f32, tag="ap_",
                                          name=f"po{qc}_{nlo}")
                        nc.tensor.matmul(
                            ps_o[:, 0:nw],
                            lhsT=ATa[:, qsl],
                            rhs=wop[:, nlo:nlo + nw],
                            start=True, stop=False,
                        )
                        nc.tensor.matmul(
                            ps_o[:, 0:nw],
                            lhsT=ATb[:, qsl],
                            rhs=wo2[:, nlo:nlo + nw],
                            start=False, stop=True,
                        )
                        if (qc + (nlo > 0)) % 2 == 0:
                            nc.vector.tensor_copy(o[:, nlo:nlo + nw],
                                                  ps_o[:, 0:nw])
                        else:
                            nc.scalar.copy(o[:, nlo:nlo + nw], ps_o[:, 0:nw])
                    nc.sync.dma_start(out=out_d[qsl, :], in_=o[:])

    nc.compile()
    return nc


def _get_program():
    with _lock:
        if "nc" not in _cache:
            _cache["nc"] = _build_program()
        return _cache["nc"]


def _host_prep(core, query, key_value, key_padding_mask, Wq, Wk, Wv, Wo, rel_emb):
    import ml_dtypes

    b, g = core // 4, core % 4
    bf = ml_dtypes.bfloat16
    mask = key_padding_mask[b].astype(np.float32)
    kv = key_value[b] * mask[:, None]
    # [128, NQ, CCH, 512]: partition p, n-slice, c-chunk (rows 128c+p)
    qTn = np.ascontiguousarray(
        query[b].T.reshape(CCH, 128, NQ, 512).transpose(1, 2, 0, 3)
    ).astype(bf)
    kvTn = np.ascontiguousarray(
        kv.T.reshape(CCH, 128, NQ, 512).transpose(1, 2, 0, 3)
    ).astype(bf)
    sl = slice(HD * g, HD * (g + 1))
    # [128, CCH, HD]: weight row 128c+p at [p, c, :]
    wq = np.ascontiguousarray(
        Wq[:, sl].reshape(CCH, 128, HD).transpose(1, 0, 2)).astype(bf)
    wk = np.ascontiguousarray(
        (Wk[:, sl] * np.float32(DK ** -0.5)).reshape(CCH, 128, HD)
        .transpose(1, 0, 2)).astype(bf)
    wv = np.ascontiguousarray(
        Wv[:, sl].reshape(CCH, 128, HD).transpose(1, 0, 2)).astype(bf)
    wo3 = Wo[sl].reshape(HP, 64, D)
    wop = np.ascontiguousarray(wo3[0:2].reshape(128, D)).astype(bf)
    wo2 = np.ascontiguousarray(wo3[2]).astype(bf)

    d = np.arange(-2047, 2048)
    buckets = _np_bucket(d)
    heads = [HP * g + i for i in range(HP)]
    t = rel_emb[buckets][:, heads].astype(np.float32)  # [4095, HP]
    cm = t[0]
    cp = t[-1]
    # sh[p, h, y] = t[y + 1793 + p, h] - cm[h]
    p_i = np.arange(128)[:, None]
    y_i = np.arange(383)[None, :]
    sh = np.ascontiguousarray(
        (t[y_i + 1793 + p_i] - cm[None, None, :]).transpose(0, 2, 1))
    msk = np.ascontiguousarray(mask.reshape(KC, 128).T)
    cmc = np.ascontiguousarray(np.broadcast_to(cm[None, :], (128, HP)))
    mu = np.exp(cp - cm).astype(np.float32)
    muc = np.ascontiguousarray(np.broadcast_to(mu[None, :], (128, HP)))
    return {
        "qTn": qTn, "kvTn": kvTn, "wq": wq, "wk": wk, "wv": wv,
        "wop": wop, "wo2": wo2,
        "sh": sh.astype(np.float32),
        "mskb": msk.astype(bf),
        "cm": cmc.astype(np.float32), "mu": muc.astype(np.float32),
    }


def make_in_maps(**inputs):
    return [_host_prep(c, **inputs) for c in range(NCORES)]


def kernel(query, key_value, key_padding_mask, Wq, Wk, Wv, Wo, rel_emb,
           _results_hook=None, _run_kwargs=None):
    from concourse.bass_utils import run_bass_kernel_spmd

    inputs = dict(query=np.asarray(query), key_value=np.asarray(key_value),
                  key_padding_mask=np.asarray(key_padding_mask),
                  Wq=np.asarray(Wq, np.float32), Wk=np.asarray(Wk, np.float32),
                  Wv=np.asarray(Wv, np.float32), Wo=np.asarray(Wo, np.float32),
                  rel_emb=np.asarray(rel_emb, np.float32))
    nc = _get_program()
    in_maps = make_in_maps(**inputs)
    res = run_bass_kernel_spmd(nc, in_maps, core_ids=list(range(NCORES)),
                               **(_run_kwargs or {}))
    if _results_hook is not None:
        _results_hook(res)
    out = np.zeros((B, L, D), np.float32)
    for c in range(NCORES):
        out[c // 4] += res.results[c]["out_p"].astype(np.float32)
    return out
